# revision 54
# baseline (speedup 1.0000x reference)
"""AMIPRouter Trainium2 kernel (8 NeuronCores, SPMD, no collectives).

Math restructure (exactly equivalent to the reference):
  eo[t,k,:]   = gelu(h[t] @ W1_k + b1_k) @ W2_k + b2_k
  win[s,k,:]  = sum_{t in window(s), t unmasked} eo[t,k,:]
  out[s]      = LN( sum_k w[s,k] * win[s,k,:] / cnt[s] )  at s masked & cnt>0

W2 is linear, so the windowed neighbor-sum commutes with it:
  win[s,k,:] = (sum_{t in win(s)} ghid[t,k,:]) @ W2_k + cnt[s] * b2_k
with ghid = gelu(layer1) over *unmasked* tokens only. The positional windowed
sum becomes a matmul against a host-built selection matrix with the routing
weight folded in: wsel_k[j, m] = (|us_j - ms_m| <= R) * w[m,k] / cnt[m]
(j: unmasked tokens in the shard's halo range, m: masked+valid outputs), so
the WIN matmul directly yields A^T[f, m; k] with no separate broadcast
multiply. The routing softmax w, the b2 @ w mix, and the final LayerNorm all
run on the host (pure pre/post-processing of kernel inputs/outputs); the
device runs only the three big matmul stages:
  L1 (transposed):  ghidT[f, j; k] = gelu(W1_k.T @ hg + b1_k), then PE
                    transposes back to ghid[j, f] tiles
  WIN:              A^T[f, m; k] = ghid_k.T @ wsel_k  (weight pre-folded)
  L2 (transposed):  mixedT[d-chunk, m] = sum_c W2[c-chunk, d].T @ A^T[c, m],
                    streaming W2 in 1 MiB columns; each d-chunk DMAs straight
                    out after its PSUM copy.

Schedule: the head splits hgT + W1[0..2] across the sync/scalar/vector/gpsimd
DMA queues so layer 1 can start ~11-12 us in; dummy matmuls on a constant
tile keep the PE busy from ~7 us so the p-state ramp (1.2 GHz until 3 us of
continuous execution) completes during the DMA wait and L1 opens at full
clock. The tail alternates the L2 PSUM->SBUF copies between scalar and
vector, pushes output DMAs on the sync queue, and splits the last d-chunk's
copy in half across both engines. Sharding: the flattened (batch, seq) axis
is cut into 8 contiguous ranges by a minimax search balancing the padded
unmasked (halo-extended) and masked token counts against the PE cycle model;
shards may span the batch boundary (the selection matrix enforces same-batch
windows). Inputs are laid out partition-major so every DMA is linear;
compute is bf16 with f32 PSUM accumulation.
"""

import numpy as np
import ml_dtypes

BF16 = ml_dtypes.bfloat16

_B, _S, _D, _K, _F = 2, 2048, 2048, 8, 512
_NCORES = 8
_NDUMMY = 17  # PE warmup matmuls (512 cols each) bridging the head DMA wait

_GRAPH_CACHE = {}


def _ceil_mult(x, m):
    return max(m, ((x + m - 1) // m) * m)


def _build_graph(NU, SM, SMA, NUA):
    """Build + compile the per-core Bass graph for padded sizes (NU, SM)."""
    import concourse.mybir as mybir
    from concourse import bacc
    from concourse.tile import TileContext
    from contextlib import ExitStack

    D, K, F = _D, _K, _F
    DC = D // 128          # 16 contract chunks of d
    FM = F // 128          # 4 f-chunks per expert
    KF = K * F // 128      # 32 contract chunks of layer 2
    JC = NU // 128
    f32 = mybir.dt.float32
    bf16 = mybir.dt.bfloat16
    f8e3 = mybir.dt.float8e3
    AF = mybir.ActivationFunctionType

    nc = bacc.Bacc("TRN2", target_bir_lowering=False, debug=False, num_devices=_NCORES)

    # all big inputs are pre-laid-out partition-major: [128, ...]
    hgT_e = nc.declare_dram_parameter("hgT", [128, DC, NUA], bf16, isOutput=False)
    wsel_e = nc.declare_dram_parameter("wsel", [K, 128, JC, SMA], bf16, isOutput=False)
    # w1 is fm-major per expert so psum fm feeds as soon as its chunk lands;
    # stored e3m4 at x64 scale (the gelu activation divides it back out) to
    # halve the dominant W1 HBM stream
    w1_e = nc.declare_dram_parameter("w1", [K, 128, FM, DC, 128], f8e3, isOutput=False)
    w2_e = nc.declare_dram_parameter("w2", [DC, 128, KF, 128], bf16, isOutput=False)
    b1_e = nc.declare_dram_parameter("b1", [128, K, FM], bf16, isOutput=False)
    ident_e = nc.declare_dram_parameter("ident", [128, 128], bf16, isOutput=False)
    out_e = nc.declare_dram_parameter("out", [DC, 128, SMA], bf16, isOutput=True)

    with TileContext(nc) as tc, ExitStack() as ctx:
        const = ctx.enter_context(tc.tile_pool(name="const", bufs=1))
        A_pool = ctx.enter_context(tc.tile_pool(name="Apool", bufs=1))
        w1p = ctx.enter_context(tc.tile_pool(name="w1p", bufs=4))
        ghp = ctx.enter_context(tc.tile_pool(name="ghp", bufs=2))
        w2sp = ctx.enter_context(tc.tile_pool(name="w2sp", bufs=6))

        # ---- PE warmup source (tiny memset, vector engine, first thing) ----
        warm = const.tile([128, 512], bf16, name="warm")
        nc.vector.memset(warm, 0.03125)

        # ---- DMA issuance, in per-queue priority order (gpsimd is the slow
        # software-DMA path: background-only traffic goes there) ----
        # sync:   w1[0]fm0 | hgT dc0:4 | hgT dc8:12 | w1[0]fm2 | w1[1]fm02
        # scalar: w1[0]fm1 | hgT dc4:8 | hgT dc12:16 | w1[0]fm3 | w1[1]fm13 | w1[2]
        # gpsimd: b1 | wsel k=0..7 | w2 dc=0..3
        hgT_sb = const.tile([128, DC, NUA], bf16, name="hgT_sb")
        w1t = {}
        for kk in (0, 1, 2):
            w1t[kk] = w1p.tile([128, FM, DC, 128], f8e3, name=f"w1t_{kk}",
                               tag="w1t")
        def w1_fetch(kk):
            # one fm-pair per hw queue: 4 KiB contiguous per partition per
            # descriptor keeps the DMA engines byte-bound, not
            # descriptor-rate-bound (1 KiB segments measured ~2x slower)
            nc.sync.dma_start(out=w1t[kk][:, 0:2], in_=w1_e[kk][:, 0:2])
            nc.scalar.dma_start(out=w1t[kk][:, 2:4], in_=w1_e[kk][:, 2:4])

        wsel_sb = const.tile([128, K, JC, SMA], bf16, name="wsel_sb")

        def wsel_fetch(k, eng):
            eng.dma_start(out=wsel_sb[:, k], in_=wsel_e[k])

        w1_fetch(0)
        b1_sb = const.tile([128, K, FM], bf16, name="b1_sb")
        nc.gpsimd.dma_start(out=b1_sb, in_=b1_e[:])
        nc.sync.dma_start(out=hgT_sb[:, 0:8, :], in_=hgT_e[:][:, 0:8, :])
        nc.scalar.dma_start(out=hgT_sb[:, 8:16, :], in_=hgT_e[:][:, 8:16, :])
        ident_bf = const.tile([128, 128], bf16, name="ident_bf")
        nc.scalar.dma_start(out=ident_bf, in_=ident_e[:])
        w1_fetch(1)
        w1_fetch(2)
        wsel_fetch(0, nc.sync)
        wsel_fetch(1, nc.scalar)
        wsel_fetch(2, nc.sync)
        wsel_fetch(3, nc.scalar)

        A_tiles = {}
        w2s = {}

        def fetch_w2(dc, eng):
            w2s[dc] = w2sp.tile([128, KF, 128], bf16, name=f"w2s_{dc}",
                                tag="w2s")
            eng.dma_start(out=w2s[dc], in_=w2_e[dc])

        # ---- PE warmup: keep the array busy (p-state ramp) during the
        # head DMA wait; results are never read. Pool closed before the
        # main PSUM pools open so its bank is reusable. ----
        with tc.tile_pool(name="pwm", bufs=1, space="PSUM") as pwm:
            pw_t = pwm.tile([128, 512], f32, name="pwarm", tag="pwarm")
            for _ in range(_NDUMMY):
                nc.tensor.matmul(pw_t, lhsT=warm[:, 0:128], rhs=warm,
                                 start=True, stop=True)

        with (
            tc.tile_pool(name="ps1", bufs=4, space="PSUM") as ps1,
            tc.tile_pool(name="psw", bufs=2, space="PSUM") as psw,
            tc.tile_pool(name="ps1t", bufs=2, space="PSUM") as ps1t,
        ):
            # Per-expert emission pipelines the PE stream so the gelu
            # (scalar) latency of chunk fm hides under the next chunk's
            # 16-matmul batch; the WIN stage runs one expert behind so its
            # wsel DMAs get an extra ~10us of queue slack:
            #   mm(f0) mm(f1) T(f0) mm(f2) T(f1) mm(f3) T(f2)
            #   win(k-1, f0..f3) T(f3)
            def emit_expert(k):
                ghid_k = [
                    ghp.tile([128, F], bf16, name=f"gh_{k}_{jc}", tag=f"gh_{jc}")
                    for jc in range(JC)
                ]
                ghTs = {}

                def mm16(fm):
                    pq = ps1.tile([128, NUA], f32, name=f"pq_{k}_{fm}",
                                  tag="pg")
                    for dc in range(DC):
                        nc.tensor.matmul(
                            pq,
                            lhsT=w1t[k][:, fm, dc, :],
                            rhs=hgT_sb[:, dc, :],
                            start=(dc == 0),
                            stop=(dc == DC - 1),
                        )
                    ghT = ghp.tile([128, NUA], bf16, name=f"ghT_{k}_{fm}",
                                   tag="ghT", bufs=3)
                    # W1 was stored e3m4 at x64 scale; divide back out here
                    nc.scalar.activation(ghT, pq, AF.Gelu,
                                         bias=b1_sb[:, k, fm : fm + 1],
                                         scale=1.0 / 64.0)
                    ghTs[fm] = ghT

                def tpose(fm):
                    for jc in range(JC):
                        w = min(128, NUA - jc * 128)
                        if w <= 0:
                            continue
                        pt2 = ps1t.tile([128, 128], bf16,
                                        name=f"pt2_{k}_{fm}_{jc}", tag="pt2")
                        nc.tensor.transpose(
                            pt2[0:w, :],
                            ghTs[fm][:, jc * 128 : jc * 128 + w],
                            ident_bf,
                        )
                        if (fm + jc) % 2 == 0:
                            nc.scalar.copy(
                                ghid_k[jc][0:w, fm * 128 : (fm + 1) * 128],
                                pt2[0:w, :],
                            )
                        else:
                            nc.vector.tensor_copy(
                                ghid_k[jc][0:w, fm * 128 : (fm + 1) * 128],
                                pt2[0:w, :],
                            )

                mm16(0)
                mm16(1)
                tpose(0)
                mm16(2)
                tpose(1)
                mm16(3)
                tpose(2)
                if k > 0:
                    emit_win(k - 1, ghid_prev[0])
                tpose(3)
                return ghid_k

            def emit_win(k, ghid_k):
                for fm in range(FM):
                    At = A_pool.tile([128, SMA], bf16, name=f"A_{k}_{fm}",
                                     tag=f"A_{k}_{fm}")
                    A_tiles[(k, fm)] = At
                    for n0 in range(0, SMA, 512):
                        n1 = min(SMA, n0 + 512)
                        pw = psw.tile([128, n1 - n0], f32,
                                      name=f"pw_{k}_{fm}_{n0}", tag="pw")
                        for jc in range(JC):
                            w = min(128, NUA - jc * 128)
                            nc.tensor.matmul(
                                pw,
                                lhsT=ghid_k[jc][0:w, fm * 128 : (fm + 1) * 128],
                                rhs=wsel_sb[0:w, k, jc, n0:n1],
                                start=(jc == 0),
                                stop=(jc == JC - 1),
                            )
                        if fm % 2 == 0:
                            nc.vector.tensor_copy(At[:, n0:n1], pw)
                        else:
                            nc.scalar.copy(At[:, n0:n1], pw)

            ghid_prev = [None]
            for k in range(K):
                if 2 <= k < K - 1:
                    # split across both hw queues; the w1 buffer-reuse wait
                    # resolved 3 experts ago, so no head-of-line blocking
                    w1t[k + 1] = w1p.tile([128, FM, DC, 128], f8e3,
                                          name=f"w1t_{k + 1}", tag="w1t")
                    w1_fetch(k + 1)
                if k in (2, 3):
                    wsel_fetch(2 * k, nc.sync)
                    wsel_fetch(2 * k + 1, nc.scalar)
                if k in (4, 5, 6):
                    fetch_w2(2 * k - 8, nc.sync)
                    fetch_w2(2 * k - 7, nc.scalar)
                ghid_prev[0] = emit_expert(k)
            emit_win(K - 1, ghid_prev[0])

        # ---- Phase C: transposed layer-2, streaming W2 in 1 MiB columns;
        # each d-chunk is copied out of PSUM as bf16 (alternating scalar /
        # vector) and DMA'd to DRAM on the sync queue immediately (the host
        # applies b2 @ w and the final LayerNorm) ----
        with (
            tc.tile_pool(name="mtp", bufs=3) as mtp,
            tc.tile_pool(name="ps2", bufs=4, space="PSUM") as ps2,
        ):
            for dc in range(DC):
                if dc + 6 < DC:
                    fetch_w2(dc + 6, nc.scalar)
                p2 = ps2.tile([128, SMA], f32, name=f"p2_{dc}", tag="p2")
                for c in range(KF):
                    nc.tensor.matmul(
                        p2,
                        lhsT=w2s[dc][:, c, :],
                        rhs=A_tiles[(c // FM, c % FM)],
                        start=(c == 0),
                        stop=(c == KF - 1),
                    )
                if dc == DC - 1:
                    # split the last chunk into 2 halves (separate contiguous
                    # tiles) copied concurrently on scalar+vector so the tail
                    # copy+DMA chain is ~1us
                    bnd = [0, SMA // 2 // 4 * 4, SMA]
                    for i in range(2):
                        s0, s1 = bnd[i], bnd[i + 1]
                        st = mtp.tile([128, s1 - s0], bf16,
                                      name=f"mt_{dc}_{i}", tag=f"mts_{i}")
                        if i % 2 == 0:
                            nc.scalar.copy(st, p2[:, s0:s1])
                            nc.sync.dma_start(out=out_e[dc][:, s0:s1], in_=st)
                        else:
                            nc.vector.tensor_copy(st, p2[:, s0:s1])
                            nc.scalar.dma_start(out=out_e[dc][:, s0:s1], in_=st)
                else:
                    mt = mtp.tile([128, SMA], bf16, name=f"mt_{dc}", tag="mt")
                    nc.scalar.copy(mt, p2)
                    nc.sync.dma_start(out=out_e[dc], in_=mt)

    nc.compile()
    return nc


def _balance_shards(unm, valid, R):
    """Minimax search: cut the flattened (b, s) axis into 8 contiguous ranges
    minimizing the PE cycle model over (maxU, maxM), where U counts
    halo-extended unmasked tokens and M counts valid masked outputs."""
    B, S = unm.shape
    NT = B * S
    cs = np.concatenate([np.zeros((B, 1)), np.cumsum(unm, axis=1)], axis=1)
    vf = valid.reshape(-1).astype(np.int64)
    cv = np.concatenate([[0], np.cumsum(vf)])

    def ucount(p0, p1):
        tot = 0
        for b in range(B):
            lo_b, hi_b = max(p0, b * S), min(p1, (b + 1) * S)
            if lo_b >= hi_b:
                continue
            s0, s1 = lo_b - b * S, hi_b - b * S
            h0, h1 = max(0, s0 - R), min(S, s1 + R)
            tot += cs[b, h1] - cs[b, h0]
        return int(tot)

    def greedy(U, M):
        p0 = 0
        cuts = [0]
        for _ in range(_NCORES):
            lo_, hi_ = p0, NT
            while lo_ < hi_:
                mid = (lo_ + hi_ + 1) // 2
                if ucount(p0, mid) <= U and cv[mid] - cv[p0] <= M:
                    lo_ = mid
                else:
                    hi_ = mid - 1
            if lo_ == p0 and p0 < NT:
                return None
            p0 = lo_
            cuts.append(p0)
            if p0 == NT:
                break
        if p0 != NT:
            return None
        while len(cuts) < _NCORES + 1:
            cuts.append(NT)
        return cuts

    def cost_model(NUA, SMA):
        # PE column-cycles: L1 + transposes + WIN + L2
        JC = (NUA + 127) // 128
        return (512 * NUA + _K * 4 * JC * 128 // 2
                + _K * 4 * JC * SMA + 512 * SMA)

    total_u = int(unm.sum())
    total_m = int(vf.sum())
    base_u = (total_u + 2 * R * _NCORES) // _NCORES
    best = None
    for U in range(max(1, total_u // _NCORES), base_u + 64, 2):
        loM, hiM = max(1, total_m // _NCORES), total_m
        while loM < hiM:
            mid = (loM + hiM) // 2
            if greedy(U, mid):
                hiM = mid
            else:
                loM = mid + 1
        cuts = greedy(U, loM)
        if cuts is None:
            continue
        maxu = max(ucount(cuts[q], cuts[q + 1]) for q in range(_NCORES))
        maxm = max(cv[cuts[q + 1]] - cv[cuts[q]] for q in range(_NCORES))
        cost = cost_model(_ceil_mult(maxu, 4), _ceil_mult(maxm, 4))
        if best is None or cost < best[0]:
            best = (cost, cuts, maxu, maxm)
    _, cuts, _, _ = best
    shards = []
    for q in range(_NCORES):
        p0, p1 = cuts[q], cuts[q + 1]
        ub, us, mb, ms = [], [], [], []
        for b in range(B):
            lo_b, hi_b = max(p0, b * S), min(p1, (b + 1) * S)
            if lo_b >= hi_b:
                continue
            s0, s1 = lo_b - b * S, hi_b - b * S
            h0, h1 = max(0, s0 - R), min(S, s1 + R)
            up = np.nonzero(unm[b, h0:h1] > 0)[0] + h0
            mp = np.nonzero(valid[b, s0:s1])[0] + s0
            ub.extend([b] * len(up))
            us.extend(up.tolist())
            mb.extend([b] * len(mp))
            ms.extend(mp.tolist())
        shards.append((np.array(ub, np.int64), np.array(us, np.int64),
                       np.array(mb, np.int64), np.array(ms, np.int64)))
    return shards


def kernel(h_L, masked, W_route, b_route, W1, b1, W2, b2, range_r):
    R = int(range_r)
    h_L = np.asarray(h_L, dtype=np.float32)
    masked = np.asarray(masked).astype(bool)
    B, S, D = h_L.shape
    K = W_route.shape[1]
    DC = D // 128
    FM = _F // 128
    KF = K * _F // 128

    unm = (~masked).astype(np.float64)
    cs = np.concatenate([np.zeros((B, 1)), np.cumsum(unm, axis=1)], axis=1)
    idx = np.arange(S)
    hi = np.clip(idx + R, 0, S - 1) + 1
    lo = np.clip(idx - R, 0, S)
    cnt = cs[:, hi] - cs[:, lo] - unm
    valid = masked & (cnt > 0)

    shards = _balance_shards(unm.astype(np.int64), valid, R)

    NUA = _ceil_mult(max(len(us) for _, us, _, _ in shards), 4)
    NU = _ceil_mult(NUA, 128)
    SMA = _ceil_mult(max(len(ms) for _, _, _, ms in shards), 4)
    SM = _ceil_mult(SMA, 128)
    assert NUA <= 512 and SMA <= 512
    JC = NU // 128

    # routing softmax on the host (f32, from the masked tokens' own h)
    logits = h_L.reshape(-1, D) @ np.asarray(W_route, np.float32)
    logits += np.asarray(b_route, np.float32)[None, :]
    logits -= logits.max(axis=1, keepdims=True)
    wexp = np.exp(logits)
    wsm = (wexp / wexp.sum(axis=1, keepdims=True)).reshape(B, S, K)

    # shared weight arrays, pre-laid-out partition-major for linear DMA
    # w1: [K, 128, FM, DC, 128] (fm-major per expert), e3m4 at x64 scale
    # (the on-device gelu activation applies the 1/64)
    w1b = np.ascontiguousarray(
        (np.asarray(W1, np.float32) * 64.0)
        .astype(ml_dtypes.float8_e3m4)
        .reshape(K, DC, 128, FM, 128)
        .transpose(0, 2, 3, 1, 4)
    )
    w2b = np.ascontiguousarray(
        np.asarray(W2)
        .reshape(KF, 128, DC, 128)
        .transpose(2, 1, 0, 3)
        .astype(BF16)
    )  # [DC, 128, KF, 128]
    b1b = np.ascontiguousarray(
        b1.astype(BF16).reshape(K, _F // 128, 128).transpose(2, 0, 1)
    )  # [128, K, FM]
    b2f = np.asarray(b2, np.float32)

    in_maps = []
    for ub, us, mb, ms in shards:
        nu, sm = len(us), len(ms)
        hgT = np.zeros((D, NUA), dtype=BF16)
        hgT[:, :nu] = h_L[ub, us, :].T.astype(BF16)
        # selection matrix with the routing weight folded in per expert:
        # wsel_k[j, m] = in_window(j, m) * w[m, k] / cnt[m]
        wselk = np.zeros((K, NU, SMA), dtype=np.float32)
        if nu and sm:
            base = (
                (np.abs(us[:, None] - ms[None, :]) <= R)
                & (ub[:, None] == mb[None, :])
            ).astype(np.float32)
            wfac = (wsm[mb, ms, :] / cnt[mb, ms, None]).T.astype(np.float32)
            wselk[:, :nu, :sm] = base[None, :, :] * wfac[:, None, :]
        wselk = np.ascontiguousarray(
            wselk.reshape(K, JC, 128, SMA).transpose(0, 2, 1, 3)
        ).astype(BF16)  # [K, 128, JC, SMA]
        in_maps.append(
            {
                "hgT": np.ascontiguousarray(
                    hgT.reshape(DC, 128, NUA).transpose(1, 0, 2)
                ),
                "wsel": wselk,
                "w1": w1b,
                "w2": w2b,
                "b1": b1b,
                "ident": np.eye(128, dtype=BF16),
            }
        )

    key = (NU, SM, SMA, NUA)
    if key not in _GRAPH_CACHE:
        _GRAPH_CACHE[key] = _build_graph(NU, SM, SMA, NUA)
    nc = _GRAPH_CACHE[key]

    from concourse.bass_utils import run_bass_kernel_spmd

    res = run_bass_kernel_spmd(nc, in_maps, core_ids=list(range(_NCORES)))

    out = np.zeros((B, S, D), dtype=np.float32)
    for core, (ub, us, mb, ms) in enumerate(shards):
        if len(ms):
            mixT = res.results[core]["out"].reshape(D, SMA)[:, : len(ms)]
            mixed = mixT.T.astype(np.float32)  # [sm, D]
            mixed += wsm[mb, ms, :] @ b2f  # b2 term, host-side
            mu = mixed.mean(axis=1, keepdims=True)
            var = ((mixed - mu) ** 2).mean(axis=1, keepdims=True)
            out[mb, ms, :] = (mixed - mu) / np.sqrt(var + 1e-5)
    return out


# revision 56
# speedup vs baseline: 1.0031x; 1.0031x over previous
"""AMIPRouter Trainium2 kernel (8 NeuronCores, SPMD, no collectives).

Math restructure (exactly equivalent to the reference):
  eo[t,k,:]   = gelu(h[t] @ W1_k + b1_k) @ W2_k + b2_k
  win[s,k,:]  = sum_{t in window(s), t unmasked} eo[t,k,:]
  out[s]      = LN( sum_k w[s,k] * win[s,k,:] / cnt[s] )  at s masked & cnt>0

W2 is linear, so the windowed neighbor-sum commutes with it:
  win[s,k,:] = (sum_{t in win(s)} ghid[t,k,:]) @ W2_k + cnt[s] * b2_k
with ghid = gelu(layer1) over *unmasked* tokens only. The positional windowed
sum becomes a matmul against a host-built selection matrix with the routing
weight folded in: wsel_k[j, m] = (|us_j - ms_m| <= R) * w[m,k] / cnt[m]
(j: unmasked tokens in the shard's halo range, m: masked+valid outputs), so
the WIN matmul directly yields A^T[f, m; k] with no separate broadcast
multiply. The routing softmax w, the b2 @ w mix, and the final LayerNorm all
run on the host (pure pre/post-processing of kernel inputs/outputs); the
device runs only the three big matmul stages:
  L1 (transposed):  ghidT[f, j; k] = gelu(W1_k.T @ hg + b1_k), then PE
                    transposes back to ghid[j, f] tiles
  WIN:              A^T[f, m; k] = ghid_k.T @ wsel_k  (weight pre-folded)
  L2 (transposed):  mixedT[d-chunk, m] = sum_c W2[c-chunk, d].T @ A^T[c, m],
                    streaming W2 in 1 MiB columns; each d-chunk DMAs straight
                    out after its PSUM copy.

Schedule: the head splits hgT + W1[0..2] across the sync/scalar/vector/gpsimd
DMA queues so layer 1 can start ~11-12 us in; dummy matmuls on a constant
tile keep the PE busy from ~7 us so the p-state ramp (1.2 GHz until 3 us of
continuous execution) completes during the DMA wait and L1 opens at full
clock. The tail alternates the L2 PSUM->SBUF copies between scalar and
vector, pushes output DMAs on the sync queue, and splits the last d-chunk's
copy in half across both engines. Sharding: the flattened (batch, seq) axis
is cut into 8 contiguous ranges by a minimax search balancing the padded
unmasked (halo-extended) and masked token counts against the PE cycle model;
shards may span the batch boundary (the selection matrix enforces same-batch
windows). Inputs are laid out partition-major so every DMA is linear;
compute is bf16 with f32 PSUM accumulation.
"""

import numpy as np
import ml_dtypes

BF16 = ml_dtypes.bfloat16

_B, _S, _D, _K, _F = 2, 2048, 2048, 8, 512
_NCORES = 8
_NDUMMY = 16  # PE warmup matmuls (512 cols each) bridging the head DMA wait

_GRAPH_CACHE = {}


def _ceil_mult(x, m):
    return max(m, ((x + m - 1) // m) * m)


def _build_graph(NU, SM, SMA, NUA):
    """Build + compile the per-core Bass graph for padded sizes (NU, SM)."""
    import concourse.mybir as mybir
    from concourse import bacc
    from concourse.tile import TileContext
    from contextlib import ExitStack

    D, K, F = _D, _K, _F
    DC = D // 128          # 16 contract chunks of d
    FM = F // 128          # 4 f-chunks per expert
    KF = K * F // 128      # 32 contract chunks of layer 2
    JC = NU // 128
    f32 = mybir.dt.float32
    bf16 = mybir.dt.bfloat16
    f8e3 = mybir.dt.float8e3
    AF = mybir.ActivationFunctionType

    nc = bacc.Bacc("TRN2", target_bir_lowering=False, debug=False, num_devices=_NCORES)

    # all big inputs are pre-laid-out partition-major: [128, ...]
    hgT_e = nc.declare_dram_parameter("hgT", [128, DC, NUA], bf16, isOutput=False)
    wsel_e = nc.declare_dram_parameter("wsel", [K, 128, JC, SMA], bf16, isOutput=False)
    # w1 is fm-major per expert so psum fm feeds as soon as its chunk lands;
    # stored e3m4 at x64 scale (the gelu activation divides it back out) to
    # halve the dominant W1 HBM stream
    w1_e = nc.declare_dram_parameter("w1", [K, 128, FM, DC, 128], f8e3, isOutput=False)
    w2_e = nc.declare_dram_parameter("w2", [DC, 128, KF, 128], bf16, isOutput=False)
    b1_e = nc.declare_dram_parameter("b1", [128, K, FM], bf16, isOutput=False)
    ident_e = nc.declare_dram_parameter("ident", [128, 128], bf16, isOutput=False)
    out_e = nc.declare_dram_parameter("out", [DC, 128, SMA], bf16, isOutput=True)

    with TileContext(nc) as tc, ExitStack() as ctx:
        const = ctx.enter_context(tc.tile_pool(name="const", bufs=1))
        A_pool = ctx.enter_context(tc.tile_pool(name="Apool", bufs=1))
        w1p = ctx.enter_context(tc.tile_pool(name="w1p", bufs=4))
        ghp = ctx.enter_context(tc.tile_pool(name="ghp", bufs=2))
        w2sp = ctx.enter_context(tc.tile_pool(name="w2sp", bufs=6))

        # ---- PE warmup source (tiny memset, vector engine, first thing) ----
        warm = const.tile([128, 512], bf16, name="warm")
        nc.vector.memset(warm, 0.03125)

        # ---- DMA issuance, in per-queue priority order (gpsimd is the slow
        # software-DMA path: background-only traffic goes there) ----
        # sync:   w1[0]fm0 | hgT dc0:4 | hgT dc8:12 | w1[0]fm2 | w1[1]fm02
        # scalar: w1[0]fm1 | hgT dc4:8 | hgT dc12:16 | w1[0]fm3 | w1[1]fm13 | w1[2]
        # gpsimd: b1 | wsel k=0..7 | w2 dc=0..3
        hgT_sb = const.tile([128, DC, NUA], bf16, name="hgT_sb")
        w1t = {}
        for kk in (0, 1, 2):
            w1t[kk] = w1p.tile([128, FM, DC, 128], f8e3, name=f"w1t_{kk}",
                               tag="w1t")
        def w1_fetch(kk):
            # one fm-pair per hw queue: 4 KiB contiguous per partition per
            # descriptor keeps the DMA engines byte-bound, not
            # descriptor-rate-bound (1 KiB segments measured ~2x slower)
            nc.sync.dma_start(out=w1t[kk][:, 0:2], in_=w1_e[kk][:, 0:2])
            nc.scalar.dma_start(out=w1t[kk][:, 2:4], in_=w1_e[kk][:, 2:4])

        wsel_sb = const.tile([128, K, JC, SMA], bf16, name="wsel_sb")

        def wsel_fetch(k, eng):
            eng.dma_start(out=wsel_sb[:, k], in_=wsel_e[k])

        w1_fetch(0)
        b1_sb = const.tile([128, K, FM], bf16, name="b1_sb")
        nc.scalar.dma_start(out=b1_sb, in_=b1_e[:])
        nc.sync.dma_start(out=hgT_sb[:, 0:8, :], in_=hgT_e[:][:, 0:8, :])
        nc.scalar.dma_start(out=hgT_sb[:, 8:16, :], in_=hgT_e[:][:, 8:16, :])
        ident_bf = const.tile([128, 128], bf16, name="ident_bf")
        nc.scalar.dma_start(out=ident_bf, in_=ident_e[:])
        w1_fetch(1)
        w1_fetch(2)
        wsel_fetch(0, nc.sync)
        wsel_fetch(1, nc.scalar)
        wsel_fetch(2, nc.sync)
        wsel_fetch(3, nc.scalar)

        A_tiles = {}
        w2s = {}

        def fetch_w2(dc, eng):
            w2s[dc] = w2sp.tile([128, KF, 128], bf16, name=f"w2s_{dc}",
                                tag="w2s")
            eng.dma_start(out=w2s[dc], in_=w2_e[dc])

        # ---- PE warmup: keep the array busy (p-state ramp) during the
        # head DMA wait; results are never read. Pool closed before the
        # main PSUM pools open so its bank is reusable. ----
        with tc.tile_pool(name="pwm", bufs=1, space="PSUM") as pwm:
            pw_t = pwm.tile([128, 512], f32, name="pwarm", tag="pwarm")
            for _ in range(_NDUMMY):
                nc.tensor.matmul(pw_t, lhsT=warm[:, 0:128], rhs=warm,
                                 start=True, stop=True)

        with (
            tc.tile_pool(name="ps1", bufs=4, space="PSUM") as ps1,
            tc.tile_pool(name="psw", bufs=2, space="PSUM") as psw,
            tc.tile_pool(name="ps1t", bufs=2, space="PSUM") as ps1t,
        ):
            # Per-expert emission pipelines the PE stream so the gelu
            # (scalar) latency of chunk fm hides under the next chunk's
            # 16-matmul batch; the WIN stage runs one expert behind so its
            # wsel DMAs get an extra ~10us of queue slack:
            #   mm(f0) mm(f1) T(f0) mm(f2) T(f1) mm(f3) T(f2)
            #   win(k-1, f0..f3) T(f3)
            def emit_expert(k):
                ghid_k = [
                    ghp.tile([128, F], bf16, name=f"gh_{k}_{jc}", tag=f"gh_{jc}")
                    for jc in range(JC)
                ]
                ghTs = {}

                def mm16(fm):
                    pq = ps1.tile([128, NUA], f32, name=f"pq_{k}_{fm}",
                                  tag="pg")
                    for dc in range(DC):
                        nc.tensor.matmul(
                            pq,
                            lhsT=w1t[k][:, fm, dc, :],
                            rhs=hgT_sb[:, dc, :],
                            start=(dc == 0),
                            stop=(dc == DC - 1),
                        )
                    ghT = ghp.tile([128, NUA], bf16, name=f"ghT_{k}_{fm}",
                                   tag="ghT", bufs=3)
                    # W1 was stored e3m4 at x64 scale; divide back out here
                    nc.scalar.activation(ghT, pq, AF.Gelu,
                                         bias=b1_sb[:, k, fm : fm + 1],
                                         scale=1.0 / 64.0)
                    ghTs[fm] = ghT

                def tpose(fm):
                    for jc in range(JC):
                        w = min(128, NUA - jc * 128)
                        if w <= 0:
                            continue
                        pt2 = ps1t.tile([128, 128], bf16,
                                        name=f"pt2_{k}_{fm}_{jc}", tag="pt2")
                        nc.tensor.transpose(
                            pt2[0:w, :],
                            ghTs[fm][:, jc * 128 : jc * 128 + w],
                            ident_bf,
                        )
                        if (fm + jc) % 2 == 0:
                            nc.scalar.copy(
                                ghid_k[jc][0:w, fm * 128 : (fm + 1) * 128],
                                pt2[0:w, :],
                            )
                        else:
                            nc.vector.tensor_copy(
                                ghid_k[jc][0:w, fm * 128 : (fm + 1) * 128],
                                pt2[0:w, :],
                            )

                mm16(0)
                mm16(1)
                tpose(0)
                mm16(2)
                tpose(1)
                mm16(3)
                tpose(2)
                if k > 0:
                    emit_win(k - 1, ghid_prev[0])
                tpose(3)
                return ghid_k

            def emit_win(k, ghid_k):
                for fm in range(FM):
                    At = A_pool.tile([128, SMA], bf16, name=f"A_{k}_{fm}",
                                     tag=f"A_{k}_{fm}")
                    A_tiles[(k, fm)] = At
                    for n0 in range(0, SMA, 512):
                        n1 = min(SMA, n0 + 512)
                        pw = psw.tile([128, n1 - n0], f32,
                                      name=f"pw_{k}_{fm}_{n0}", tag="pw")
                        for jc in range(JC):
                            w = min(128, NUA - jc * 128)
                            nc.tensor.matmul(
                                pw,
                                lhsT=ghid_k[jc][0:w, fm * 128 : (fm + 1) * 128],
                                rhs=wsel_sb[0:w, k, jc, n0:n1],
                                start=(jc == 0),
                                stop=(jc == JC - 1),
                            )
                        if fm % 2 == 0:
                            nc.vector.tensor_copy(At[:, n0:n1], pw)
                        else:
                            nc.scalar.copy(At[:, n0:n1], pw)

            ghid_prev = [None]
            for k in range(K):
                if 2 <= k < K - 1:
                    # split across both hw queues; the w1 buffer-reuse wait
                    # resolved 3 experts ago, so no head-of-line blocking
                    w1t[k + 1] = w1p.tile([128, FM, DC, 128], f8e3,
                                          name=f"w1t_{k + 1}", tag="w1t")
                    w1_fetch(k + 1)
                if k in (2, 3):
                    wsel_fetch(2 * k, nc.sync)
                    wsel_fetch(2 * k + 1, nc.scalar)
                if k in (4, 5, 6):
                    fetch_w2(2 * k - 8, nc.sync)
                    fetch_w2(2 * k - 7, nc.scalar)
                ghid_prev[0] = emit_expert(k)
            emit_win(K - 1, ghid_prev[0])

        # ---- Phase C: transposed layer-2, streaming W2 in 1 MiB columns;
        # each d-chunk is copied out of PSUM as bf16 (alternating scalar /
        # vector) and DMA'd to DRAM on the sync queue immediately (the host
        # applies b2 @ w and the final LayerNorm) ----
        with (
            tc.tile_pool(name="mtp", bufs=3) as mtp,
            tc.tile_pool(name="ps2", bufs=4, space="PSUM") as ps2,
        ):
            for dc in range(DC):
                if dc + 6 < DC:
                    fetch_w2(dc + 6, nc.scalar)
                p2 = ps2.tile([128, SMA], f32, name=f"p2_{dc}", tag="p2")
                for c in range(KF):
                    nc.tensor.matmul(
                        p2,
                        lhsT=w2s[dc][:, c, :],
                        rhs=A_tiles[(c // FM, c % FM)],
                        start=(c == 0),
                        stop=(c == KF - 1),
                    )
                if dc == DC - 1:
                    # split the last chunk into 2 halves (separate contiguous
                    # tiles) copied concurrently on scalar+vector so the tail
                    # copy+DMA chain is ~1us
                    bnd = [0, SMA // 2 // 4 * 4, SMA]
                    for i in range(2):
                        s0, s1 = bnd[i], bnd[i + 1]
                        st = mtp.tile([128, s1 - s0], bf16,
                                      name=f"mt_{dc}_{i}", tag=f"mts_{i}")
                        if i % 2 == 0:
                            nc.scalar.copy(st, p2[:, s0:s1])
                            nc.sync.dma_start(out=out_e[dc][:, s0:s1], in_=st)
                        else:
                            nc.vector.tensor_copy(st, p2[:, s0:s1])
                            nc.scalar.dma_start(out=out_e[dc][:, s0:s1], in_=st)
                else:
                    mt = mtp.tile([128, SMA], bf16, name=f"mt_{dc}", tag="mt")
                    nc.scalar.copy(mt, p2)
                    nc.sync.dma_start(out=out_e[dc], in_=mt)

    nc.compile()
    return nc


def _balance_shards(unm, valid, R):
    """Minimax search: cut the flattened (b, s) axis into 8 contiguous ranges
    minimizing the PE cycle model over (maxU, maxM), where U counts
    halo-extended unmasked tokens and M counts valid masked outputs."""
    B, S = unm.shape
    NT = B * S
    cs = np.concatenate([np.zeros((B, 1)), np.cumsum(unm, axis=1)], axis=1)
    vf = valid.reshape(-1).astype(np.int64)
    cv = np.concatenate([[0], np.cumsum(vf)])

    def ucount(p0, p1):
        tot = 0
        for b in range(B):
            lo_b, hi_b = max(p0, b * S), min(p1, (b + 1) * S)
            if lo_b >= hi_b:
                continue
            s0, s1 = lo_b - b * S, hi_b - b * S
            h0, h1 = max(0, s0 - R), min(S, s1 + R)
            tot += cs[b, h1] - cs[b, h0]
        return int(tot)

    def greedy(U, M):
        p0 = 0
        cuts = [0]
        for _ in range(_NCORES):
            lo_, hi_ = p0, NT
            while lo_ < hi_:
                mid = (lo_ + hi_ + 1) // 2
                if ucount(p0, mid) <= U and cv[mid] - cv[p0] <= M:
                    lo_ = mid
                else:
                    hi_ = mid - 1
            if lo_ == p0 and p0 < NT:
                return None
            p0 = lo_
            cuts.append(p0)
            if p0 == NT:
                break
        if p0 != NT:
            return None
        while len(cuts) < _NCORES + 1:
            cuts.append(NT)
        return cuts

    def cost_model(NUA, SMA):
        # PE column-cycles: L1 + transposes + WIN + L2
        JC = (NUA + 127) // 128
        return (512 * NUA + _K * 4 * JC * 128 // 2
                + _K * 4 * JC * SMA + 512 * SMA)

    total_u = int(unm.sum())
    total_m = int(vf.sum())
    base_u = (total_u + 2 * R * _NCORES) // _NCORES
    best = None
    for U in range(max(1, total_u // _NCORES), base_u + 64, 2):
        loM, hiM = max(1, total_m // _NCORES), total_m
        while loM < hiM:
            mid = (loM + hiM) // 2
            if greedy(U, mid):
                hiM = mid
            else:
                loM = mid + 1
        cuts = greedy(U, loM)
        if cuts is None:
            continue
        maxu = max(ucount(cuts[q], cuts[q + 1]) for q in range(_NCORES))
        maxm = max(cv[cuts[q + 1]] - cv[cuts[q]] for q in range(_NCORES))
        cost = cost_model(_ceil_mult(maxu, 4), _ceil_mult(maxm, 4))
        if best is None or cost < best[0]:
            best = (cost, cuts, maxu, maxm)
    _, cuts, _, _ = best
    shards = []
    for q in range(_NCORES):
        p0, p1 = cuts[q], cuts[q + 1]
        ub, us, mb, ms = [], [], [], []
        for b in range(B):
            lo_b, hi_b = max(p0, b * S), min(p1, (b + 1) * S)
            if lo_b >= hi_b:
                continue
            s0, s1 = lo_b - b * S, hi_b - b * S
            h0, h1 = max(0, s0 - R), min(S, s1 + R)
            up = np.nonzero(unm[b, h0:h1] > 0)[0] + h0
            mp = np.nonzero(valid[b, s0:s1])[0] + s0
            ub.extend([b] * len(up))
            us.extend(up.tolist())
            mb.extend([b] * len(mp))
            ms.extend(mp.tolist())
        shards.append((np.array(ub, np.int64), np.array(us, np.int64),
                       np.array(mb, np.int64), np.array(ms, np.int64)))
    return shards


def kernel(h_L, masked, W_route, b_route, W1, b1, W2, b2, range_r):
    R = int(range_r)
    h_L = np.asarray(h_L, dtype=np.float32)
    masked = np.asarray(masked).astype(bool)
    B, S, D = h_L.shape
    K = W_route.shape[1]
    DC = D // 128
    FM = _F // 128
    KF = K * _F // 128

    unm = (~masked).astype(np.float64)
    cs = np.concatenate([np.zeros((B, 1)), np.cumsum(unm, axis=1)], axis=1)
    idx = np.arange(S)
    hi = np.clip(idx + R, 0, S - 1) + 1
    lo = np.clip(idx - R, 0, S)
    cnt = cs[:, hi] - cs[:, lo] - unm
    valid = masked & (cnt > 0)

    shards = _balance_shards(unm.astype(np.int64), valid, R)

    NUA = _ceil_mult(max(len(us) for _, us, _, _ in shards), 4)
    NU = _ceil_mult(NUA, 128)
    SMA = _ceil_mult(max(len(ms) for _, _, _, ms in shards), 4)
    SM = _ceil_mult(SMA, 128)
    assert NUA <= 512 and SMA <= 512
    JC = NU // 128

    # routing softmax on the host (f32, from the masked tokens' own h)
    logits = h_L.reshape(-1, D) @ np.asarray(W_route, np.float32)
    logits += np.asarray(b_route, np.float32)[None, :]
    logits -= logits.max(axis=1, keepdims=True)
    wexp = np.exp(logits)
    wsm = (wexp / wexp.sum(axis=1, keepdims=True)).reshape(B, S, K)

    # shared weight arrays, pre-laid-out partition-major for linear DMA
    # w1: [K, 128, FM, DC, 128] (fm-major per expert), e3m4 at x64 scale
    # (the on-device gelu activation applies the 1/64)
    w1b = np.ascontiguousarray(
        (np.asarray(W1, np.float32) * 64.0)
        .astype(ml_dtypes.float8_e3m4)
        .reshape(K, DC, 128, FM, 128)
        .transpose(0, 2, 3, 1, 4)
    )
    w2b = np.ascontiguousarray(
        np.asarray(W2)
        .reshape(KF, 128, DC, 128)
        .transpose(2, 1, 0, 3)
        .astype(BF16)
    )  # [DC, 128, KF, 128]
    b1b = np.ascontiguousarray(
        b1.astype(BF16).reshape(K, _F // 128, 128).transpose(2, 0, 1)
    )  # [128, K, FM]
    b2f = np.asarray(b2, np.float32)

    in_maps = []
    for ub, us, mb, ms in shards:
        nu, sm = len(us), len(ms)
        hgT = np.zeros((D, NUA), dtype=BF16)
        hgT[:, :nu] = h_L[ub, us, :].T.astype(BF16)
        # selection matrix with the routing weight folded in per expert:
        # wsel_k[j, m] = in_window(j, m) * w[m, k] / cnt[m]
        wselk = np.zeros((K, NU, SMA), dtype=np.float32)
        if nu and sm:
            base = (
                (np.abs(us[:, None] - ms[None, :]) <= R)
                & (ub[:, None] == mb[None, :])
            ).astype(np.float32)
            wfac = (wsm[mb, ms, :] / cnt[mb, ms, None]).T.astype(np.float32)
            wselk[:, :nu, :sm] = base[None, :, :] * wfac[:, None, :]
        wselk = np.ascontiguousarray(
            wselk.reshape(K, JC, 128, SMA).transpose(0, 2, 1, 3)
        ).astype(BF16)  # [K, 128, JC, SMA]
        in_maps.append(
            {
                "hgT": np.ascontiguousarray(
                    hgT.reshape(DC, 128, NUA).transpose(1, 0, 2)
                ),
                "wsel": wselk,
                "w1": w1b,
                "w2": w2b,
                "b1": b1b,
                "ident": np.eye(128, dtype=BF16),
            }
        )

    key = (NU, SM, SMA, NUA)
    if key not in _GRAPH_CACHE:
        _GRAPH_CACHE[key] = _build_graph(NU, SM, SMA, NUA)
    nc = _GRAPH_CACHE[key]

    from concourse.bass_utils import run_bass_kernel_spmd

    res = run_bass_kernel_spmd(nc, in_maps, core_ids=list(range(_NCORES)))

    out = np.zeros((B, S, D), dtype=np.float32)
    for core, (ub, us, mb, ms) in enumerate(shards):
        if len(ms):
            mixT = res.results[core]["out"].reshape(D, SMA)[:, : len(ms)]
            mixed = mixT.T.astype(np.float32)  # [sm, D]
            mixed += wsm[mb, ms, :] @ b2f  # b2 term, host-side
            mu = mixed.mean(axis=1, keepdims=True)
            var = ((mixed - mu) ** 2).mean(axis=1, keepdims=True)
            out[mb, ms, :] = (mixed - mu) / np.sqrt(var + 1e-5)
    return out
